# revision 27
# baseline (speedup 1.0000x reference)
"""Causal self-attention on 8 NeuronCores (Bass/Tile).

Sharding: tensor-parallel over heads x data-parallel over batch.
  core c -> batch b = c//4, heads 4g..4g+3 where g = c%4.
Each core computes q,k,v for its 4 heads (over its batch's 2048 tokens),
causal softmax attention, and the partial output projection over its 256
head-channels. Host sums the 4 partials per batch and adds b_proj.

v3 design: fp8(e4m3) DoubleRow matmuls for the q/k projection chains and
the score matmuls (cost model: DoubleRow fp8 = 0.5 cyc/row with 2x128
contraction per instruction -> 4x cheaper qk projection, 2x cheaper
scores). Numerics (measured vs f32 reference): ~1.65e-2 max-rel, under
the 2e-2 gate. v/pv/proj stay bf16 (fp8 there fails the gate).

Layout for fp8 scores: per head the contraction is d=64, split as
[32 partitions x 2 DoubleRow sub-rows]. q/k are stored as two tile sets:
  tile A: heads 0,1 at partition offsets 0,32 (the direct drain target)
  tile B: heads 2,3, DMA-shifted from A's partitions 64-127 down to 0-63
(PE matmuls with lhsT/rhs partition base 64/96 fail BIR/runtime; SBUF->
SBUF DMA moves across partitions instead). The qk psum chains emit the
channel order c = 64*(p//32) + 32*i + p%32 via host-side W column
permutation, so each drain stays partition-aligned. Drains are DVE
tensor_scalar (psum * QS/(XS*WS) + QS*bias -> fp8), with exp scale
1/QS^2 folded into the ACT activation.

With PE cut to ~65 us the Activation engine (exp: ~58 us of elements +
per-instr bubbles) becomes the critical engine; emission keeps ACT fed:
scores are emitted just-in-time ahead of their exps, and all bf16 PE
work (v chains, pv, transpose, proj) + qk chains ride as filler between
score slots. Masks run on GPSIMD(Pool), off the DVE/ACT critical paths.
"""

import os
import sys

for _p in ("/opt/trn_rl_repo", "/opt/pypackages"):
    if os.path.isdir(_p) and _p not in sys.path:
        sys.path.append(_p)

import numpy as np

import concourse.bass as bass
import concourse.tile as tile
import concourse.mybir as mybir
from concourse import bacc
from concourse.bass_utils import run_bass_kernel_spmd

B, T, C = 2, 2048, 1024
H = 16            # total heads
D = 64            # head dim
HPC = 4           # heads per core
CH = HPC * D      # 256 channels per core
N_CORES = 8

f32 = mybir.dt.float32
bf16 = mybir.dt.bfloat16
fp8 = mybir.dt.float8e4
ts = bass.ts
ds = bass.ds
AF = mybir.ActivationFunctionType
ALU = mybir.AluOpType
PM = mybir.MatmulPerfMode

XS = 8.0    # fp8 x pre-scale
WS = 64.0   # fp8 W pre-scale
QS = 2.0    # stored q/k fp8 scale
RS = 8.0    # x residual scale (RS*WS = XS*WS -> uniform psum scale)
SS = 64.0   # Wv residual scale (XS*SS = XS*WS)
DRAIN_S = float(QS / (XS * WS))
EXP_S = float(1.0 / (QS * QS))

_COMPILED = None

POLICY = {
    "norm_lag": int(os.environ.get("K_NORM_LAG", "1")),
    "spend_proj": os.environ.get("K_SPEND_PROJ", "s3"),  # none|s3|s23|all
    "sc_bufs": int(os.environ.get("K_SC_BUFS", "2")),
    "big_bufs": int(os.environ.get("K_BIG_BUFS", "2")),
    "masks_pool": int(os.environ.get("K_MASKS_POOL", "1")),
    "v3_defer": int(os.environ.get("K_V3_DEFER", "1")),
    "out_bf16": int(os.environ.get("K_OUT_BF16", "1")),
    "adv_fill": int(os.environ.get("K_ADV_FILL", "1")),
    "py_bufs": int(os.environ.get("K_PY_BUFS", "2")),
    "xt_prefetch": int(os.environ.get("K_XT_PREFETCH", "1")),
    "drain_alt": int(os.environ.get("K_DRAIN_ALT", "2")),
    "tail_fine": int(os.environ.get("K_TAIL_FINE", "1")),
    "proj_merge": int(os.environ.get("K_PROJ_MERGE", "1")),
    "v_defer_all": int(os.environ.get("K_V_DEFER_ALL", "1")),
    "double_fill": int(os.environ.get("K_DOUBLE_FILL", "0")),
    "slot_swap": int(os.environ.get("K_SLOT_SWAP", "0")),
    "fp8_scores": int(os.environ.get("K_FP8_SCORES", "1")),  # fallback knob
    "warm_exp": int(os.environ.get("K_WARM_EXP", "1")),
    "bulk_q": os.environ.get("K_BULK_Q", "sp"),  # act|sp: bulk DMA queue
    "pv_split": int(os.environ.get("K_PV_SPLIT", "0")),
    "act_drain0": int(os.environ.get("K_ACT_DRAIN0", "2")),
    "warmup": int(os.environ.get("K_WARMUP", "10")),
    "v_pair": int(os.environ.get("K_V_PAIR", "1")),
    "dma_t": int(os.environ.get("K_DMA_T", "0")),
}


def _build():
    nc = bacc.Bacc("TRN2", target_bir_lowering=False, debug=False,
                   num_devices=N_CORES)

    # DRAM inputs (host-prepped layouts)
    x8 = nc.dram_tensor("x8", [128, 4, 2, T], fp8, kind="ExternalInput").ap()
    xw8 = nc.dram_tensor("xw8", [128, 4, 2, 1024], fp8,
                         kind="ExternalInput").ap()
    if POLICY["v_pair"]:
        r8 = nc.dram_tensor("r8", [128, 4, 2, T], fp8,
                            kind="ExternalInput").ap()
        wv8 = nc.dram_tensor("wv8", [128, 4, 2, 2 * CH], fp8,
                             kind="ExternalInput").ap()
    else:
        xT = nc.dram_tensor("xT", [C, T], bf16, kind="ExternalInput").ap()
        wv = nc.dram_tensor("wv", [128, 8, CH], bf16,
                            kind="ExternalInput").ap()
    wpt = nc.dram_tensor("wpt", [CH, C], bf16, kind="ExternalInput").ap()
    bqk = nc.dram_tensor("bqk", [128, 4], f32, kind="ExternalInput").ap()
    bvb = nc.dram_tensor("bvb", [128, HPC, D], f32, kind="ExternalInput").ap()
    Sm = nc.dram_tensor("Sm", [128, 2, 128], bf16, kind="ExternalInput").ap()
    Idm = nc.dram_tensor("Idm", [128, 128], bf16, kind="ExternalInput").ap()
    out_dt = bf16 if POLICY["out_bf16"] else f32
    out = nc.dram_tensor("out_partial", [T, C], out_dt,
                         kind="ExternalOutput").ap()

    NT512 = T // 512          # 4   512-token stripes
    NT128 = T // 128          # 16  128-token tiles
    NC128 = C // 128          # 8   contraction tiles (bf16 v path)

    with tile.TileContext(nc) as tc:
        with tc.tile_pool(name="consts", bufs=1) as consts, \
             tc.tile_pool(name="qkv", bufs=1) as qkv, \
             tc.tile_pool(name="x8p",
                          bufs=POLICY["xt_prefetch"] + 1) as x8p, \
             tc.tile_pool(name="xp",
                          bufs=POLICY["xt_prefetch"] + 1) as xp, \
             tc.tile_pool(name="pp", bufs=17) as pp, \
             tc.tile_pool(name="yn", bufs=2) as yn, \
             tc.tile_pool(name="op", bufs=3) as op, \
             tc.tile_pool(name="ps_s", bufs=POLICY["sc_bufs"],
                          space="PSUM") as ps_s, \
             tc.tile_pool(name="ps_y", bufs=POLICY["py_bufs"],
                          space="PSUM") as ps_y, \
             tc.tile_pool(name="ps_big", bufs=POLICY["big_bufs"],
                          space="PSUM") as ps_big:

            bulk = nc.scalar if POLICY["bulk_q"] == "act" else nc.sync

            # ---- startup DMAs on the SP queue: bias first (tiny), then a
            #      small j=0 pair so the first DR chain matmul starts early,
            #      then the remaining chunks as two bigger transfers ----
            bqk_sb = consts.tile([128, 4], f32)
            nc.sync.dma_start(bqk_sb[:], bqk)
            xw_sb = consts.tile([128, 4, 2, 1024], fp8)
            for j in range(4):
                nc.sync.dma_start(xw_sb[:, j], xw8[:, j])

            # dummy exp: pulls LoadActFuncSet into the startup DMA window
            if POLICY["warm_exp"]:
                warm = consts.tile([128, 1], f32)
                nc.vector.memset(warm[:], 0.0)
                warm_o = consts.tile([128, 1], bf16)
                nc.scalar.activation(warm_o[:], warm[:], AF.Exp)

            if POLICY["warmup"]:
                # dummy matmuls: keep PE continuously busy through the
                # startup DMA wait so the p-state ramp completes before the
                # first real chains (and they run at full clock).
                wz = consts.tile([128, 512], bf16)
                nc.gpsimd.memset(wz[:], 0.0)
                for _ in range(POLICY["warmup"]):
                    wps = ps_big.tile([128, 512], f32, tag="big")
                    nc.tensor.matmul(wps[:], wz[:, :128], wz[:],
                                     start=True, stop=True)

            vp = POLICY["v_pair"]
            if vp:
                wv8_sb = consts.tile([128, 4, 2, 2 * CH], fp8)
            else:
                xT_r = xT.rearrange("(o p) t -> p o t", p=128)
                xt0 = xp.tile([128, NC128, 512], bf16, tag="xt")
                wv_sb = consts.tile([128, 8, CH], bf16)
            bvb_sb = consts.tile([128, HPC, D], f32)
            S_sb = consts.tile([128, 2, 128], bf16)
            Id_sb = consts.tile([128, 128], bf16)
            wpt_sb = consts.tile([128, 2, C], bf16)

            def emit_bulk_loads():
                # emitted after stripe-0 qk units so their shift DMAs get
                # transfer-priority; ordered by first-use time
                bulk.dma_start(S_sb[:], Sm)
                ensure_xt_dma(1, x8_only=True)
                if vp:
                    ensure_r8_dma(0)
                    bulk.dma_start(wv8_sb[:], wv8)
                else:
                    bulk.dma_start(xt0[:], xT_r[:, :, ts(0, 512)])
                    bulk.dma_start(wv_sb[:], wv)
                bulk.dma_start(bvb_sb[:], bvb)
                bulk.dma_start(Id_sb[:], Idm)
                ensure_xt_dma(1)
                bulk.dma_start(
                    wpt_sb[:], wpt.rearrange("(s p) o -> p s o", p=128))

            # ---- persistent activations ----
            # q/k fp8: tile A holds heads 0,1 (parts 0..63) as drained;
            # B gets heads 2,3 DMA-shifted from A's parts 64..127.
            qT8a = qkv.tile([128, 2, T], fp8)
            qT8b = qkv.tile([128, 2, T], fp8)
            kT8a = qkv.tile([128, 2, T], fp8)
            kT8b = qkv.tile([128, 2, T], fp8)
            vaug = qkv.tile([128, NT128, HPC, D + 1], bf16)  # [kt,ki,h,d|1]
            yT = qkv.tile([128, 2, T], bf16)

            nc.vector.memset(vaug[:, :, :, D:D + 1], 1.0)

            # ---------------- emission helpers ----------------
            from collections import deque

            x8_tiles = {}
            xt_tiles = {} if POLICY["v_pair"] else {0: xt0}
            r8_tiles = {}

            def ensure_r8_dma(ti):
                if ti < NT512 and ti not in r8_tiles:
                    r8t = xp.tile([128, 4, 2, 512], fp8, tag="r8t")
                    nc.sync.dma_start(r8t[:], r8[:, :, :, ts(ti, 512)])
                    r8_tiles[ti] = r8t

            def ensure_xt_dma(ti, x8_only=False):
                if 0 < ti < NT512 and ti not in x8_tiles:
                    x8t = x8p.tile([128, 4, 2, 512], fp8, tag="x8t")
                    nc.sync.dma_start(x8t[:], x8[:, :, :, ts(ti, 512)])
                    x8_tiles[ti] = x8t
                if x8_only:
                    return
                if POLICY["v_pair"]:
                    ensure_r8_dma(ti)
                elif ti < NT512 and ti not in xt_tiles:
                    xt = xp.tile([128, NC128, 512], bf16, tag="xt")
                    nc.sync.dma_start(xt[:], xT_r[:, :, ts(ti, 512)])
                    xt_tiles[ti] = xt

            # chain cc: 0=q,i0  1=q,i1  2=k,i0  3=k,i1
            def emit_qk_chain(ti, cc):
                st = qT8a if cc < 2 else kT8a
                i = cc % 2
                ps = ps_big.tile([128, 512], f32, tag="big")
                for j in range(4):
                    if ti == 0:
                        wsl = xw_sb[:, j, :, ts(cc, 128)]
                        xsl = xw_sb[:, j, :, ds(512, 512)]
                    else:
                        wsl = xw_sb[:, j, :, ts(cc, 128)]
                        xsl = x8_tiles[ti][:, j]
                    nc.tensor.matmul(
                        ps[:], wsl, xsl,
                        start=(j == 0), stop=(j == 3),
                        perf_mode=PM.DoubleRow)
                if ti <= POLICY["act_drain0"] - 1 and cc in (0, 2):
                    # startup: ACT is idle; halve the serial drain latency
                    nc.scalar.activation(
                        st[:, i, ts(ti, 512)], ps[:], AF.Identity,
                        bias=bqk_sb[:, cc:cc + 1], scale=DRAIN_S)
                else:
                    nc.vector.tensor_scalar(
                        st[:, i, ts(ti, 512)], ps[:], DRAIN_S,
                        bqk_sb[:, cc:cc + 1], op0=ALU.mult, op1=ALU.add)

            def emit_qk_shift(ti, qk):
                # move heads 2,3 (parts 64..127) down to parts 0..63 of B
                a, b = (qT8a, qT8b) if qk == 0 else (kT8a, kT8b)
                nc.sync.dma_start(b[0:64, :, ts(ti, 512)],
                                  a[ds(64, 64), :, ts(ti, 512)])

            def qk_units(ti):
                return [(emit_qk_chain, (ti, 0)),
                        (emit_qk_chain, (ti, 1)),
                        (emit_qk_shift, (ti, 0)),
                        (emit_qk_chain, (ti, 2)),
                        (emit_qk_chain, (ti, 3)),
                        (emit_qk_shift, (ti, 1))]

            def emit_v_chain(ti, tj):
                pv = ps_big.tile([128, HPC, D], f32, tag="big")
                if POLICY["v_pair"]:
                    # v = (x8@w8 + x8@s8 + r8@w8) / 512 + bv  (DR fp8 pairs)
                    x8t = (xw_sb[:, :, :, ds(512, 512)] if ti == 0
                           else x8_tiles[ti])
                    r8t = r8_tiles[ti]
                    k = 0
                    for lhs, c0 in ((x8t, 0), (x8t, CH), (r8t, 0)):
                        for j in range(4):
                            nc.tensor.matmul(
                                pv[:, :, :], lhs[:, j, :, ts(tj, 128)],
                                wv8_sb[:, j, :, ds(c0, CH)],
                                start=(k == 0), stop=(k == 11),
                                perf_mode=PM.DoubleRow,
                                skip_group_check=True)
                            k += 1
                    nc.vector.scalar_tensor_tensor(
                        vaug[:, 4 * ti + tj, :, 0:D], pv[:, :, :],
                        1.0 / (XS * WS), bvb_sb[:],
                        op0=ALU.mult, op1=ALU.add)
                    return
                xt = xt_tiles[ti]
                for ci in range(NC128):
                    nc.tensor.matmul(
                        pv[:, :, :], xt[:, ci, ts(tj, 128)],
                        wv_sb[:, ci, :],
                        start=(ci == 0), stop=(ci == NC128 - 1))
                nc.vector.tensor_add(
                    out=vaug[:, 4 * ti + tj, :, 0:D],
                    in0=pv[:, :, :], in1=bvb_sb[:])

            p4_all = {}  # (stripe, ki) -> p4 tile

            def emit_scores(qi, ki, groups=(0, 1)):
                j = ki - 4 * qi
                q0 = max(0, 128 * j)
                w = 512 - q0
                if (qi, ki) in p4_all:
                    p4 = p4_all[(qi, ki)]
                else:
                    p4 = pp.tile([128, HPC, 512], bf16, tag="p4")
                for g in groups:
                    KT = kT8a if g == 0 else kT8b
                    QT = qT8a if g == 0 else qT8b
                    sc = ps_s.tile([128, 2, 512], f32, tag="sc")
                    for hh in range(2):
                        nc.tensor.matmul(
                            sc[:, hh, q0:],
                            KT[ts(hh, 32), :, ts(ki, 128)],
                            QT[ts(hh, 32), :, ds(512 * qi + q0, w)],
                            start=True, stop=True,
                            perf_mode=PM.DoubleRow,
                            tile_position=(32 * hh, 0))
                    nc.scalar.activation(
                        p4[:, ts(g, 2), q0:], sc[:, :, q0:], AF.Exp,
                        scale=EXP_S)
                    if j >= 0:
                        # last diag feeds the serial tail: use DVE (faster op,
                        # keeps Pool queueing off the critical path)
                        eng = (nc.vector if (ki == NT128 - 1
                                             or not POLICY["masks_pool"])
                               else nc.gpsimd)
                        eng.tensor_mul(
                            out=p4[:, ts(g, 2), q0:q0 + 128],
                            in0=p4[:, ts(g, 2), q0:q0 + 128],
                            in1=S_sb[:])
                p4_all[(qi, ki)] = p4

            def emit_pv(tg, last_ki=None):
                # last_ki < tg leaves the accumulation groups open; a later
                # emit_pv_fin() adds the remaining k-blocks and closes them.
                tg_rel, qi = tg % 4, tg // 4
                if last_ki is None:
                    last_ki = tg
                py4 = ps_y.tile([128, HPC, D + 1], f32, tag="py")
                sp = POLICY["pv_split"] == 1
                for h in range(HPC):
                    for ki in range(last_ki + 1):
                        # one group per tile when split: a start=True re-marks
                        # the WHOLE psum bank pending-zero, which would wipe
                        # other heads' partial accumulations across slots
                        nc.tensor.matmul(
                            py4[:, h, :],
                            p4_all[(qi, ki)][:, h, ts(tg_rel, 128)],
                            vaug[:, ki, h, :],
                            start=((h == 0 if sp else True) and ki == 0),
                            stop=(not sp and ki == tg),
                            skip_group_check=sp)
                py4s[tg] = py4
                pv_done[tg] = last_ki
                if not POLICY["norm_lag"] and last_ki == tg:
                    emit_norm(tg)

            def emit_pv_fin(tg):
                tg_rel, qi = tg % 4, tg // 4
                py4 = py4s[tg]
                sp = POLICY["pv_split"] == 1
                for h in range(HPC):
                    for ki in range(pv_done[tg] + 1, tg + 1):
                        nc.tensor.matmul(
                            py4[:, h, :],
                            p4_all[(qi, ki)][:, h, ts(tg_rel, 128)],
                            vaug[:, ki, h, :],
                            start=(h == 0 and ki == 0),
                            stop=(sp and h == HPC - 1 and ki == tg),
                            skip_group_check=sp)
                pv_done[tg] = tg

            pv_done = [None] * NT128

            def emit_norm(tg):
                py4 = py4s[tg]
                rec4 = yn.tile([128, HPC, 1], f32, tag="rec")
                nc.vector.reciprocal(rec4[:], py4[:, :, D:D + 1])
                y_n = yn.tile([128, HPC, D], bf16, tag="yn")
                nc.vector.tensor_mul(
                    out=y_n[:], in0=py4[:, :, 0:D],
                    in1=rec4.to_broadcast([128, HPC, D]))
                y_ns[tg] = y_n

            py4s = [None] * NT128
            y_ns = [None] * NT128

            def emit_transpose(tg):
                if POLICY["dma_t"] and tg < NT128 - 2:
                    for i in range(2):
                        nc.sync.dma_start_transpose(
                            yT[:, i, ts(tg, 128)], y_ns[tg][:, ts(i, 2), :])
                    proj_q.append(tg)
                    return
                yTt = ps_y.tile([128, 2, 128], bf16, tag="py")
                for i in range(2):
                    nc.tensor.transpose(
                        yTt[:, i, :], y_ns[tg][:, ts(i, 2), :], Id_sb[:])
                nc.vector.tensor_copy(yT[:, :, ts(tg, 128)], yTt[:])
                proj_q.append(tg)

            def emit_proj(tg, split_drain=False):
                if POLICY["proj_merge"]:
                    pos2 = [ps_big.tile([128, 512], f32, tag="big",
                                        name=f"po_m{k}") for k in range(2)]
                    for oi in range(2):
                        for s in range(2):
                            nc.tensor.matmul(
                                pos2[oi][:], yT[:, s, ts(tg, 128)],
                                wpt_sb[:, s, ts(oi, 512)],
                                start=(s == 0), stop=(s == 1))
                    ot2 = op.tile([128, 2, 512], out_dt, tag="ot2")
                    nc.vector.tensor_copy(ot2[:, 0, :], pos2[0][:])
                    if split_drain and POLICY["drain_alt"] in (1, 2):
                        nc.scalar.activation(ot2[:, 1, :], pos2[1][:], AF.Copy)
                    else:
                        nc.vector.tensor_copy(ot2[:, 1, :], pos2[1][:])
                    nc.sync.dma_start(out[ts(tg, 128), :], ot2[:])
                    return
                for oi in range(2):
                    po = ps_big.tile([128, 512], f32, tag="big")
                    for s in range(2):
                        nc.tensor.matmul(
                            po[:], yT[:, s, ts(tg, 128)],
                            wpt_sb[:, s, ts(oi, 512)],
                            start=(s == 0), stop=(s == 1))
                    ot = op.tile([128, 512], out_dt, tag="ot")
                    nc.vector.tensor_copy(ot[:], po[:])
                    nc.sync.dma_start(out[ts(tg, 128), ts(oi, 512)], ot[:])

            # -------- software-pipelined emission --------
            filler = deque()   # pending PE-heavy units
            proj_q = deque()   # proj tiles ready to emit
            state = {"pv": 0}

            def advance_pipeline(upto, spend_proj=False):
                nl = POLICY["norm_lag"]
                sp = POLICY["pv_split"]
                while state["pv"] <= min(upto, NT128 - 1):
                    tg = state["pv"]
                    if sp:
                        if tg >= 1:
                            if sp == 1:
                                emit_pv_fin(tg - 1)
                            emit_norm(tg - 1)
                        if tg >= 2:
                            emit_transpose(tg - 2)
                        if spend_proj and proj_q:
                            emit_proj(proj_q.popleft())
                        emit_pv(tg, last_ki=(tg - 1 if sp == 1 else tg))
                    else:
                        if nl and tg >= 1:
                            emit_norm(tg - 1)
                        if tg >= 1 + nl:
                            emit_transpose(tg - 1 - nl)
                        if spend_proj and proj_q:
                            emit_proj(proj_q.popleft())
                        emit_pv(tg)
                    state["pv"] += 1

            for ti in range(NT512):
                if ti == 0:
                    for fn, args in qk_units(0):
                        fn(*args)
                    emit_bulk_loads()
                ensure_xt_dma(ti)
                for pf in range(1, POLICY["xt_prefetch"] + 1):
                    ensure_xt_dma(ti + pf)
                if ti == 3 and POLICY["v3_defer"]:
                    pass  # v(3) was queued during stripe 2
                elif POLICY["v_defer_all"]:
                    for tj in range(3, -1, -1):
                        filler.appendleft((emit_v_chain, (ti, tj)))
                else:
                    for tj in range(4):
                        emit_v_chain(ti, tj)
                if ti + 1 < NT512:
                    for unit in qk_units(ti + 1):
                        filler.append(unit)
                    if ti + 1 == 3 and POLICY["v3_defer"]:
                        # v(3) as late stripe-2 filler: frees stripe-3 PE
                        # for the diag endgame
                        for tj in range(4):
                            filler.append((emit_v_chain, (ti + 1, tj)))
                if POLICY["adv_fill"] and ti > 0:
                    units = [(advance_pipeline, (m,))
                             for m in range(state["pv"], 4 * ti)]
                    for u in reversed(units):
                        filler.appendleft(u)
                else:
                    advance_pipeline(4 * ti - 1)

                nk = 4 * ti + 4
                spend = POLICY["spend_proj"]
                nfill = 1 + (POLICY["double_fill"] and ti == 3)
                for ki in range(nk):
                    def _slot_work():
                        if ki - 1 >= 4 * ti:
                            if ti == 0 and filler:
                                fn, args = filler.popleft()
                                fn(*args)
                            advance_pipeline(
                                ki - 1,
                                spend_proj=(spend == "all"
                                            or (spend in ("s3", "s23")
                                                and ti == 3)
                                            or (spend == "s23" and ti == 2)))
                        else:
                            for _ in range(nfill):
                                if filler:
                                    fn, args = filler.popleft()
                                    fn(*args)
                                elif proj_q:
                                    emit_proj(proj_q.popleft())
                    if POLICY["slot_swap"]:
                        _slot_work()
                        emit_scores(ti, ki)
                    else:
                        emit_scores(ti, ki)
                        _slot_work()
                while filler:
                    fn, args = filler.popleft()
                    fn(*args)

            def emit_tail_fine(fin=False):
                # tile 15: per-slab fin -> norm -> transpose -> yT copy
                # interleaved with the proj chain; slab 0 overlaps the g=1
                # exp+mask of the final diagonal
                tg = NT128 - 1
                py4 = py4s[tg]
                y_n = yn.tile([128, HPC, D], bf16, tag="yn")
                yTt = ps_y.tile([128, 2, 128], bf16, tag="py")
                pos = [ps_big.tile([128, 512], f32, tag="big", name=f"po_t{k}")
                       for k in range(2)]
                tg_rel, qi = tg % 4, tg // 4
                for s in range(2):
                    if fin:
                        for h in (2 * s, 2 * s + 1):
                            for ki in range(pv_done[tg] + 1, tg + 1):
                                nc.tensor.matmul(
                                    py4[:, h, :],
                                    p4_all[(qi, ki)][:, h, ts(tg_rel, 128)],
                                    vaug[:, ki, h, :],
                                    start=False,
                                    stop=(h == HPC - 1 and ki == tg),
                                    skip_group_check=True)
                    rec2 = yn.tile([128, 2, 1], f32, tag="rec")
                    nc.vector.reciprocal(
                        rec2[:], py4[:, ts(s, 2), D:D + 1])
                    nc.vector.tensor_mul(
                        out=y_n[:, ts(s, 2), :], in0=py4[:, ts(s, 2), 0:D],
                        in1=rec2.to_broadcast([128, 2, D]))
                    nc.tensor.transpose(
                        yTt[:, s, :], y_n[:, ts(s, 2), :], Id_sb[:])
                    nc.vector.tensor_copy(
                        yT[:, s, ts(tg, 128)], yTt[:, s, :])
                    for oi in range(2):
                        nc.tensor.matmul(
                            pos[oi][:], yT[:, s, ts(tg, 128)],
                            wpt_sb[:, s, ts(oi, 512)],
                            start=(s == 0), stop=(s == 1))
                ot2 = op.tile([128, 2, 512], out_dt, tag="ot2")
                nc.vector.tensor_copy(ot2[:, 0, :], pos[0][:])
                if POLICY["drain_alt"] in (1, 2):
                    nc.scalar.activation(ot2[:, 1, :], pos[1][:], AF.Copy)
                else:
                    nc.vector.tensor_copy(ot2[:, 1, :], pos[1][:])
                nc.sync.dma_start(out[ts(tg, 128), :], ot2[:])

            advance_pipeline(NT128 - 1)
            if POLICY["pv_split"]:
                # state: pv(15) open at ki<=14, norm done <=14, transp <=13
                emit_transpose(NT128 - 2)
                while proj_q:
                    emit_proj(proj_q.popleft(),
                              split_drain=POLICY["drain_alt"] == 2)
                if POLICY["tail_fine"]:
                    emit_tail_fine(fin=True)
                else:
                    emit_pv_fin(NT128 - 1)
                    emit_norm(NT128 - 1)
                    emit_transpose(NT128 - 1)
                    emit_proj(NT128 - 1,
                              split_drain=POLICY["drain_alt"] == 2)
            elif POLICY["norm_lag"]:
                if POLICY["tail_fine"]:
                    emit_transpose(NT128 - 2)
                    while proj_q:
                        emit_proj(proj_q.popleft(),
                                  split_drain=POLICY["drain_alt"] == 2)
                    emit_tail_fine()
                else:
                    emit_norm(NT128 - 1)
                    emit_transpose(NT128 - 2)
            if not POLICY["pv_split"] and \
                    not (POLICY["norm_lag"] and POLICY["tail_fine"]):
                emit_transpose(NT128 - 1)
                while proj_q:
                    emit_proj(proj_q.popleft(),
                              split_drain=POLICY["drain_alt"] == 2)

    nc.compile()
    return nc


def _get_compiled():
    global _COMPILED
    if _COMPILED is None:
        _COMPILED = _build()
    return _COMPILED


def _host_prep(x, W_attn, b_attn, W_proj, b_proj):
    import ml_dtypes
    scale = 1.0 / np.sqrt(np.float32(D))
    xTb = [np.ascontiguousarray(x[b].T).astype(np.float32) for b in range(B)]
    Sm = (np.arange(128, dtype=np.int32)[None, :]
          >= np.arange(128, dtype=np.int32)[:, None]).astype(np.float32)
    Idm = np.eye(128, dtype=np.float32)

    # channel order for a qk chain with sub-row i: psum partition p' holds
    # local channel c = 64*(p'//32) + 32*i + (p'%32)
    pp_ = np.arange(128)
    c_of_p = {i: 64 * (pp_ // 32) + 32 * i + (pp_ % 32) for i in (0, 1)}

    in_maps = []
    for c in range(N_CORES):
        b, g = divmod(c, 4)
        ch = slice(CH * g, CH * (g + 1))
        Wq = W_attn[ch]                    # [256, C]
        Wk = W_attn[C:][ch] * scale
        Wv = W_attn[2 * C:][ch]
        bq = b_attn[ch]
        bk = b_attn[C:][ch] * scale
        bv = b_attn[2 * C:][ch]

        # x8: [128, 4(j), 2(i), T] = xT[(2j+i)*128+p, t] * XS
        x8_c = np.ascontiguousarray(
            (xTb[b].reshape(4, 2, 128, T).transpose(2, 0, 1, 3) * XS)
        ).astype(ml_dtypes.float8_e4m3)

        # xw8: [128(p), 4(j), 2(i_row), 1024] = wqk cols (512) | x8 stripe0
        # chain cc: 0=q,i0 1=q,i1 2=k,i0 3=k,i1; col p' -> channel c_of_p
        wqk = np.empty((C, 4, 128), dtype=np.float32)  # [row, chain, col]
        wqk[:, 0, :] = Wq[c_of_p[0]].T
        wqk[:, 1, :] = Wq[c_of_p[1]].T
        wqk[:, 2, :] = Wk[c_of_p[0]].T
        wqk[:, 3, :] = Wk[c_of_p[1]].T
        wqk = (wqk.reshape(4, 2, 128, 4 * 128).transpose(2, 0, 1, 3)
               * WS).astype(np.float32)
        xw8_c = np.ascontiguousarray(np.concatenate(
            [wqk, x8_c[:, :, :, :512].astype(np.float32)], axis=3)
        ).astype(ml_dtypes.float8_e4m3)

        # bqk: [128, 4] = QS * bias[channel(p', chain)]
        bqk_c = np.empty((128, 4), dtype=np.float32)
        bqk_c[:, 0] = QS * bq[c_of_p[0]]
        bqk_c[:, 1] = QS * bq[c_of_p[1]]
        bqk_c[:, 2] = QS * bk[c_of_p[0]]
        bqk_c[:, 3] = QS * bk[c_of_p[1]]

        bvb_c = np.ascontiguousarray(
            np.broadcast_to(bv[None, :].reshape(1, HPC, D),
                            (128, HPC, D))).astype(np.float32)
        wpt_c = np.ascontiguousarray(
            W_proj[:, ch].T).astype(ml_dtypes.bfloat16)

        im = {
            "x8": x8_c,
            "xw8": xw8_c,
            "wpt": wpt_c,
            "bqk": bqk_c,
            "bvb": bvb_c,
            "Sm": np.ascontiguousarray(np.broadcast_to(
                Sm[:, None, :], (128, 2, 128))).astype(ml_dtypes.bfloat16),
            "Idm": Idm.astype(ml_dtypes.bfloat16),
        }
        if POLICY["v_pair"]:
            # r8: fp8 residual of x at 8x the x8 scale; layout like x8
            x_f = xTb[b].reshape(4, 2, 128, T).transpose(2, 0, 1, 3)
            x8_f = x8_c.astype(np.float32)
            r8_c = np.ascontiguousarray(
                (x_f - x8_f / XS) * RS).astype(ml_dtypes.float8_e4m3)
            # wv8: [128, 4(j), 2(i), 512] = w8 cols (256) | s8 cols (256)
            wvt = Wv.T  # [C, 256]
            w8 = (wvt * WS).astype(ml_dtypes.float8_e4m3)
            s8 = ((wvt - w8.astype(np.float32) / WS) * SS
                  ).astype(ml_dtypes.float8_e4m3)
            wv8_f = np.concatenate(
                [w8.astype(np.float32), s8.astype(np.float32)], axis=1)
            wv8_c = np.ascontiguousarray(
                wv8_f.reshape(4, 2, 128, 2 * CH).transpose(2, 0, 1, 3)
            ).astype(ml_dtypes.float8_e4m3)
            im["r8"] = r8_c
            im["wv8"] = wv8_c
        else:
            im["xT"] = xTb[b].astype(ml_dtypes.bfloat16)
            im["wv"] = np.ascontiguousarray(
                Wv.T.reshape(8, 128, CH).transpose(1, 0, 2)
            ).astype(ml_dtypes.bfloat16)
        in_maps.append(im)
    return in_maps


def kernel(x, W_attn, b_attn, W_proj, b_proj):
    x = np.asarray(x, dtype=np.float32)
    W_attn = np.asarray(W_attn, dtype=np.float32)
    b_attn = np.asarray(b_attn, dtype=np.float32)
    W_proj = np.asarray(W_proj, dtype=np.float32)
    b_proj = np.asarray(b_proj, dtype=np.float32)

    nc = _get_compiled()
    in_maps = _host_prep(x, W_attn, b_attn, W_proj, b_proj)
    res = run_bass_kernel_spmd(nc, in_maps, core_ids=list(range(N_CORES)))

    out = np.empty((B, T, C), dtype=np.float32)
    for b in range(B):
        acc = np.asarray(res.results[4 * b]["out_partial"],
                         dtype=np.float32).copy()
        for g in range(1, 4):
            acc += np.asarray(res.results[4 * b + g]["out_partial"],
                              dtype=np.float32)
        out[b] = acc + b_proj
    return out


# revision 28
# speedup vs baseline: 1.0094x; 1.0094x over previous
"""Causal self-attention on 8 NeuronCores (Bass/Tile).

Sharding: tensor-parallel over heads x data-parallel over batch.
  core c -> batch b = c//4, heads 4g..4g+3 where g = c%4.
Each core computes q,k,v for its 4 heads (over its batch's 2048 tokens),
causal softmax attention, and the partial output projection over its 256
head-channels. Host sums the 4 partials per batch and adds b_proj.

v3 design: fp8(e4m3) DoubleRow matmuls for the q/k projection chains and
the score matmuls (cost model: DoubleRow fp8 = 0.5 cyc/row with 2x128
contraction per instruction -> 4x cheaper qk projection, 2x cheaper
scores). Numerics (measured vs f32 reference): ~1.65e-2 max-rel, under
the 2e-2 gate. v/pv/proj stay bf16 (fp8 there fails the gate).

Layout for fp8 scores: per head the contraction is d=64, split as
[32 partitions x 2 DoubleRow sub-rows]. q/k are stored as two tile sets:
  tile A: heads 0,1 at partition offsets 0,32 (the direct drain target)
  tile B: heads 2,3, DMA-shifted from A's partitions 64-127 down to 0-63
(PE matmuls with lhsT/rhs partition base 64/96 fail BIR/runtime; SBUF->
SBUF DMA moves across partitions instead). The qk psum chains emit the
channel order c = 64*(p//32) + 32*i + p%32 via host-side W column
permutation, so each drain stays partition-aligned. Drains are DVE
tensor_scalar (psum * QS/(XS*WS) + QS*bias -> fp8), with exp scale
1/QS^2 folded into the ACT activation.

With PE cut to ~65 us the Activation engine (exp: ~58 us of elements +
per-instr bubbles) becomes the critical engine; emission keeps ACT fed:
scores are emitted just-in-time ahead of their exps, and all bf16 PE
work (v chains, pv, transpose, proj) + qk chains ride as filler between
score slots. Masks run on GPSIMD(Pool), off the DVE/ACT critical paths.
"""

import os
import sys

for _p in ("/opt/trn_rl_repo", "/opt/pypackages"):
    if os.path.isdir(_p) and _p not in sys.path:
        sys.path.append(_p)

import numpy as np

import concourse.bass as bass
import concourse.tile as tile
import concourse.mybir as mybir
from concourse import bacc
from concourse.bass_utils import run_bass_kernel_spmd

B, T, C = 2, 2048, 1024
H = 16            # total heads
D = 64            # head dim
HPC = 4           # heads per core
CH = HPC * D      # 256 channels per core
N_CORES = 8

f32 = mybir.dt.float32
bf16 = mybir.dt.bfloat16
fp8 = mybir.dt.float8e4
ts = bass.ts
ds = bass.ds
AF = mybir.ActivationFunctionType
ALU = mybir.AluOpType
PM = mybir.MatmulPerfMode

XS = 8.0    # fp8 x pre-scale
WS = 64.0   # fp8 W pre-scale
QS = 2.0    # stored q/k fp8 scale
RS = 8.0    # x residual scale (RS*WS = XS*WS -> uniform psum scale)
SS = 64.0   # Wv residual scale (XS*SS = XS*WS)
DRAIN_S = float(QS / (XS * WS))
EXP_S = float(1.0 / (QS * QS))

_COMPILED = None

POLICY = {
    "norm_lag": int(os.environ.get("K_NORM_LAG", "1")),
    "spend_proj": os.environ.get("K_SPEND_PROJ", "s3"),  # none|s3|s23|all
    "sc_bufs": int(os.environ.get("K_SC_BUFS", "2")),
    "big_bufs": int(os.environ.get("K_BIG_BUFS", "2")),
    "masks_pool": int(os.environ.get("K_MASKS_POOL", "1")),
    "v3_defer": int(os.environ.get("K_V3_DEFER", "1")),
    "out_bf16": int(os.environ.get("K_OUT_BF16", "1")),
    "adv_fill": int(os.environ.get("K_ADV_FILL", "1")),
    "py_bufs": int(os.environ.get("K_PY_BUFS", "2")),
    "xt_prefetch": int(os.environ.get("K_XT_PREFETCH", "1")),
    "drain_alt": int(os.environ.get("K_DRAIN_ALT", "2")),
    "tail_fine": int(os.environ.get("K_TAIL_FINE", "1")),
    "proj_merge": int(os.environ.get("K_PROJ_MERGE", "1")),
    "v_defer_all": int(os.environ.get("K_V_DEFER_ALL", "1")),
    "double_fill": int(os.environ.get("K_DOUBLE_FILL", "0")),
    "slot_swap": int(os.environ.get("K_SLOT_SWAP", "0")),
    "fp8_scores": int(os.environ.get("K_FP8_SCORES", "1")),  # fallback knob
    "warm_exp": int(os.environ.get("K_WARM_EXP", "1")),
    "bulk_q": os.environ.get("K_BULK_Q", "sp"),  # act|sp: bulk DMA queue
    "pv_split": int(os.environ.get("K_PV_SPLIT", "0")),
    "act_drain0": int(os.environ.get("K_ACT_DRAIN0", "2")),
    "warmup": int(os.environ.get("K_WARMUP", "10")),
    "v_pair": int(os.environ.get("K_V_PAIR", "1")),
    "dma_t": int(os.environ.get("K_DMA_T", "0")),
}


def _build():
    nc = bacc.Bacc("TRN2", target_bir_lowering=False, debug=False,
                   num_devices=N_CORES)

    # DRAM inputs (host-prepped layouts)
    x8 = nc.dram_tensor("x8", [128, 4, 2, T], fp8, kind="ExternalInput").ap()
    xw8 = nc.dram_tensor("xw8", [128, 4, 2, 1024], fp8,
                         kind="ExternalInput").ap()
    if POLICY["v_pair"]:
        r8 = nc.dram_tensor("r8", [128, 4, 2, T], fp8,
                            kind="ExternalInput").ap()
        wv8 = nc.dram_tensor("wv8", [128, 4, 2, 2 * CH], fp8,
                             kind="ExternalInput").ap()
    else:
        xT = nc.dram_tensor("xT", [C, T], bf16, kind="ExternalInput").ap()
        wv = nc.dram_tensor("wv", [128, 8, CH], bf16,
                            kind="ExternalInput").ap()
    wpt = nc.dram_tensor("wpt", [CH, C], bf16, kind="ExternalInput").ap()
    bqk = nc.dram_tensor("bqk", [128, 4], f32, kind="ExternalInput").ap()
    bvb = nc.dram_tensor("bvb", [128, HPC, D], f32, kind="ExternalInput").ap()
    Sm = nc.dram_tensor("Sm", [128, 2, 128], bf16, kind="ExternalInput").ap()
    Idm = nc.dram_tensor("Idm", [128, 128], bf16, kind="ExternalInput").ap()
    out_dt = bf16 if POLICY["out_bf16"] else f32
    out = nc.dram_tensor("out_partial", [T, C], out_dt,
                         kind="ExternalOutput").ap()

    NT512 = T // 512          # 4   512-token stripes
    NT128 = T // 128          # 16  128-token tiles
    NC128 = C // 128          # 8   contraction tiles (bf16 v path)

    with tile.TileContext(nc) as tc:
        with tc.tile_pool(name="consts", bufs=1) as consts, \
             tc.tile_pool(name="qkv", bufs=1) as qkv, \
             tc.tile_pool(name="x8p",
                          bufs=POLICY["xt_prefetch"] + 1) as x8p, \
             tc.tile_pool(name="xp",
                          bufs=POLICY["xt_prefetch"] + 1) as xp, \
             tc.tile_pool(name="pp", bufs=17) as pp, \
             tc.tile_pool(name="yn", bufs=2) as yn, \
             tc.tile_pool(name="op", bufs=3) as op, \
             tc.tile_pool(name="ps_s", bufs=POLICY["sc_bufs"],
                          space="PSUM") as ps_s, \
             tc.tile_pool(name="ps_y", bufs=POLICY["py_bufs"],
                          space="PSUM") as ps_y, \
             tc.tile_pool(name="ps_big", bufs=POLICY["big_bufs"],
                          space="PSUM") as ps_big:

            bulk = nc.scalar if POLICY["bulk_q"] == "act" else nc.sync

            # ---- startup DMAs on the SP queue: bias first (tiny), then a
            #      small j=0 pair so the first DR chain matmul starts early,
            #      then the remaining chunks as two bigger transfers ----
            bqk_sb = consts.tile([128, 4], f32)
            nc.sync.dma_start(bqk_sb[:], bqk)
            xw_sb = consts.tile([128, 4, 2, 1024], fp8)
            for j in range(4):
                nc.sync.dma_start(xw_sb[:, j], xw8[:, j])

            # dummy exp: pulls LoadActFuncSet into the startup DMA window
            if POLICY["warm_exp"]:
                warm = consts.tile([128, 1], f32)
                nc.vector.memset(warm[:], 0.0)
                warm_o = consts.tile([128, 1], bf16)
                nc.scalar.activation(warm_o[:], warm[:], AF.Exp)

            if POLICY["warmup"]:
                # dummy matmuls: keep PE continuously busy through the
                # startup DMA wait so the p-state ramp completes before the
                # first real chains (and they run at full clock).
                wz = consts.tile([128, 512], bf16)
                nc.gpsimd.memset(wz[:], 0.0)
                for _ in range(POLICY["warmup"]):
                    wps = ps_big.tile([128, 512], f32, tag="big")
                    nc.tensor.matmul(wps[:], wz[:, :128], wz[:],
                                     start=True, stop=True)

            vp = POLICY["v_pair"]
            if vp:
                wv8_sb = consts.tile([128, 4, 2, 2 * CH], fp8)
            else:
                xT_r = xT.rearrange("(o p) t -> p o t", p=128)
                xt0 = xp.tile([128, NC128, 512], bf16, tag="xt")
                wv_sb = consts.tile([128, 8, CH], bf16)
            bvb_sb = consts.tile([128, HPC, D], f32)
            S_sb = consts.tile([128, 2, 128], bf16)
            Id_sb = consts.tile([128, 128], bf16)
            wpt_sb = consts.tile([128, 2, C], bf16)

            def emit_bulk_loads():
                # emitted after stripe-0 qk units so their shift DMAs get
                # transfer-priority; ordered by first-use time
                bulk.dma_start(S_sb[:], Sm)
                ensure_xt_dma(1, x8_only=True)
                if vp:
                    ensure_r8_dma(0)
                    bulk.dma_start(wv8_sb[:], wv8)
                else:
                    bulk.dma_start(xt0[:], xT_r[:, :, ts(0, 512)])
                    bulk.dma_start(wv_sb[:], wv)
                bulk.dma_start(bvb_sb[:], bvb)
                bulk.dma_start(Id_sb[:], Idm)
                ensure_xt_dma(1)
                bulk.dma_start(
                    wpt_sb[:], wpt.rearrange("(s p) o -> p s o", p=128))

            # ---- persistent activations ----
            # q/k fp8: tile A holds heads 0,1 (parts 0..63) as drained;
            # B gets heads 2,3 DMA-shifted from A's parts 64..127.
            qT8a = qkv.tile([128, 2, T], fp8)
            qT8b = qkv.tile([128, 2, T], fp8)
            kT8a = qkv.tile([128, 2, T], fp8)
            kT8b = qkv.tile([128, 2, T], fp8)
            vaug = qkv.tile([128, NT128, HPC, D + 1], bf16)  # [kt,ki,h,d|1]
            yT = qkv.tile([128, 2, T], bf16)

            nc.vector.memset(vaug[:, :, :, D:D + 1], 1.0)

            # ---------------- emission helpers ----------------
            from collections import deque

            x8_tiles = {}
            xt_tiles = {} if POLICY["v_pair"] else {0: xt0}
            r8_tiles = {}

            def ensure_r8_dma(ti):
                if ti < NT512 and ti not in r8_tiles:
                    r8t = xp.tile([128, 4, 2, 512], fp8, tag="r8t")
                    nc.sync.dma_start(r8t[:], r8[:, :, :, ts(ti, 512)])
                    r8_tiles[ti] = r8t

            def ensure_xt_dma(ti, x8_only=False):
                if 0 < ti < NT512 and ti not in x8_tiles:
                    x8t = x8p.tile([128, 4, 2, 512], fp8, tag="x8t")
                    nc.sync.dma_start(x8t[:], x8[:, :, :, ts(ti, 512)])
                    x8_tiles[ti] = x8t
                if x8_only:
                    return
                if POLICY["v_pair"]:
                    ensure_r8_dma(ti)
                elif ti < NT512 and ti not in xt_tiles:
                    xt = xp.tile([128, NC128, 512], bf16, tag="xt")
                    nc.sync.dma_start(xt[:], xT_r[:, :, ts(ti, 512)])
                    xt_tiles[ti] = xt

            # chain cc: 0=q,i0  1=q,i1  2=k,i0  3=k,i1
            def emit_qk_chain(ti, cc):
                st = qT8a if cc < 2 else kT8a
                i = cc % 2
                ps = ps_big.tile([128, 512], f32, tag="big")
                for j in range(4):
                    if ti == 0:
                        wsl = xw_sb[:, j, :, ts(cc, 128)]
                        xsl = xw_sb[:, j, :, ds(512, 512)]
                    else:
                        wsl = xw_sb[:, j, :, ts(cc, 128)]
                        xsl = x8_tiles[ti][:, j]
                    nc.tensor.matmul(
                        ps[:], wsl, xsl,
                        start=(j == 0), stop=(j == 3),
                        perf_mode=PM.DoubleRow)
                if ti <= POLICY["act_drain0"] - 1 and cc in (0, 2):
                    # startup: ACT is idle; halve the serial drain latency
                    nc.scalar.activation(
                        st[:, i, ts(ti, 512)], ps[:], AF.Identity,
                        bias=bqk_sb[:, cc:cc + 1], scale=DRAIN_S)
                else:
                    nc.vector.tensor_scalar(
                        st[:, i, ts(ti, 512)], ps[:], DRAIN_S,
                        bqk_sb[:, cc:cc + 1], op0=ALU.mult, op1=ALU.add)

            def emit_qk_shift(ti, qk):
                # move heads 2,3 (parts 64..127) down to parts 0..63 of B
                a, b = (qT8a, qT8b) if qk == 0 else (kT8a, kT8b)
                nc.sync.dma_start(b[0:64, :, ts(ti, 512)],
                                  a[ds(64, 64), :, ts(ti, 512)])

            def qk_units(ti):
                return [(emit_qk_chain, (ti, 0)),
                        (emit_qk_chain, (ti, 1)),
                        (emit_qk_shift, (ti, 0)),
                        (emit_qk_chain, (ti, 2)),
                        (emit_qk_chain, (ti, 3)),
                        (emit_qk_shift, (ti, 1))]

            def emit_v_chain(ti, tj):
                pv = ps_big.tile([128, HPC, D], f32, tag="big")
                if POLICY["v_pair"]:
                    # v = (x8@w8 + x8@s8 + r8@w8) / 512 + bv  (DR fp8 pairs)
                    x8t = (xw_sb[:, :, :, ds(512, 512)] if ti == 0
                           else x8_tiles[ti])
                    r8t = r8_tiles[ti]
                    k = 0
                    for lhs, c0 in ((x8t, 0), (x8t, CH), (r8t, 0)):
                        for j in range(4):
                            nc.tensor.matmul(
                                pv[:, :, :], lhs[:, j, :, ts(tj, 128)],
                                wv8_sb[:, j, :, ds(c0, CH)],
                                start=(k == 0), stop=(k == 11),
                                perf_mode=PM.DoubleRow,
                                skip_group_check=True)
                            k += 1
                    nc.vector.scalar_tensor_tensor(
                        vaug[:, 4 * ti + tj, :, 0:D], pv[:, :, :],
                        1.0 / (XS * WS), bvb_sb[:],
                        op0=ALU.mult, op1=ALU.add)
                    return
                xt = xt_tiles[ti]
                for ci in range(NC128):
                    nc.tensor.matmul(
                        pv[:, :, :], xt[:, ci, ts(tj, 128)],
                        wv_sb[:, ci, :],
                        start=(ci == 0), stop=(ci == NC128 - 1))
                nc.vector.tensor_add(
                    out=vaug[:, 4 * ti + tj, :, 0:D],
                    in0=pv[:, :, :], in1=bvb_sb[:])

            p4_all = {}  # (stripe, ki) -> p4 tile

            def emit_scores(qi, ki, groups=(0, 1)):
                j = ki - 4 * qi
                q0 = max(0, 128 * j)
                w = 512 - q0
                if (qi, ki) in p4_all:
                    p4 = p4_all[(qi, ki)]
                else:
                    p4 = pp.tile([128, HPC, 512], bf16, tag="p4")
                for g in groups:
                    KT = kT8a if g == 0 else kT8b
                    QT = qT8a if g == 0 else qT8b
                    sc = ps_s.tile([128, 2, 512], f32, tag="sc")
                    for hh in range(2):
                        nc.tensor.matmul(
                            sc[:, hh, q0:],
                            KT[ts(hh, 32), :, ts(ki, 128)],
                            QT[ts(hh, 32), :, ds(512 * qi + q0, w)],
                            start=True, stop=True,
                            perf_mode=PM.DoubleRow,
                            tile_position=(32 * hh, 0))
                    nc.scalar.activation(
                        p4[:, ts(g, 2), q0:], sc[:, :, q0:], AF.Exp,
                        scale=EXP_S)
                    if j >= 0:
                        # last diag feeds the serial tail: use DVE (faster op,
                        # keeps Pool queueing off the critical path)
                        eng = (nc.vector if (ki == NT128 - 1
                                             or not POLICY["masks_pool"])
                               else nc.gpsimd)
                        eng.tensor_mul(
                            out=p4[:, ts(g, 2), q0:q0 + 128],
                            in0=p4[:, ts(g, 2), q0:q0 + 128],
                            in1=S_sb[:])
                p4_all[(qi, ki)] = p4

            def emit_pv(tg, last_ki=None):
                # last_ki < tg leaves the accumulation groups open; a later
                # emit_pv_fin() adds the remaining k-blocks and closes them.
                tg_rel, qi = tg % 4, tg // 4
                if last_ki is None:
                    last_ki = tg
                py4 = ps_y.tile([128, HPC, D + 1], f32, tag="py")
                sp = POLICY["pv_split"] == 1
                for h in range(HPC):
                    for ki in range(last_ki + 1):
                        # one group per tile when split: a start=True re-marks
                        # the WHOLE psum bank pending-zero, which would wipe
                        # other heads' partial accumulations across slots
                        nc.tensor.matmul(
                            py4[:, h, :],
                            p4_all[(qi, ki)][:, h, ts(tg_rel, 128)],
                            vaug[:, ki, h, :],
                            start=((h == 0 if sp else True) and ki == 0),
                            stop=(not sp and ki == tg),
                            skip_group_check=sp)
                py4s[tg] = py4
                pv_done[tg] = last_ki
                if not POLICY["norm_lag"] and last_ki == tg:
                    emit_norm(tg)

            def emit_pv_fin(tg):
                tg_rel, qi = tg % 4, tg // 4
                py4 = py4s[tg]
                sp = POLICY["pv_split"] == 1
                for h in range(HPC):
                    for ki in range(pv_done[tg] + 1, tg + 1):
                        nc.tensor.matmul(
                            py4[:, h, :],
                            p4_all[(qi, ki)][:, h, ts(tg_rel, 128)],
                            vaug[:, ki, h, :],
                            start=(h == 0 and ki == 0),
                            stop=(sp and h == HPC - 1 and ki == tg),
                            skip_group_check=sp)
                pv_done[tg] = tg

            pv_done = [None] * NT128

            def emit_norm(tg):
                py4 = py4s[tg]
                rec4 = yn.tile([128, HPC, 1], f32, tag="rec")
                nc.vector.reciprocal(rec4[:], py4[:, :, D:D + 1])
                y_n = yn.tile([128, HPC, D], bf16, tag="yn")
                nc.vector.tensor_mul(
                    out=y_n[:], in0=py4[:, :, 0:D],
                    in1=rec4.to_broadcast([128, HPC, D]))
                y_ns[tg] = y_n

            py4s = [None] * NT128
            y_ns = [None] * NT128

            def emit_transpose(tg):
                if POLICY["dma_t"] and tg < NT128 - 2:
                    for i in range(2):
                        nc.sync.dma_start_transpose(
                            yT[:, i, ts(tg, 128)], y_ns[tg][:, ts(i, 2), :])
                    proj_q.append(tg)
                    return
                yTt = ps_y.tile([128, 2, 128], bf16, tag="py")
                for i in range(2):
                    nc.tensor.transpose(
                        yTt[:, i, :], y_ns[tg][:, ts(i, 2), :], Id_sb[:])
                nc.vector.tensor_copy(yT[:, :, ts(tg, 128)], yTt[:])
                proj_q.append(tg)

            def emit_proj(tg, split_drain=False):
                if POLICY["proj_merge"]:
                    pos2 = [ps_big.tile([128, 512], f32, tag="big",
                                        name=f"po_m{k}") for k in range(2)]
                    for oi in range(2):
                        for s in range(2):
                            nc.tensor.matmul(
                                pos2[oi][:], yT[:, s, ts(tg, 128)],
                                wpt_sb[:, s, ts(oi, 512)],
                                start=(s == 0), stop=(s == 1))
                    ot2 = op.tile([128, 2, 512], out_dt, tag="ot2")
                    nc.vector.tensor_copy(ot2[:, 0, :], pos2[0][:])
                    if split_drain and POLICY["drain_alt"] in (1, 2):
                        nc.scalar.activation(ot2[:, 1, :], pos2[1][:], AF.Copy)
                    else:
                        nc.vector.tensor_copy(ot2[:, 1, :], pos2[1][:])
                    nc.sync.dma_start(out[ts(tg, 128), :], ot2[:])
                    return
                for oi in range(2):
                    po = ps_big.tile([128, 512], f32, tag="big")
                    for s in range(2):
                        nc.tensor.matmul(
                            po[:], yT[:, s, ts(tg, 128)],
                            wpt_sb[:, s, ts(oi, 512)],
                            start=(s == 0), stop=(s == 1))
                    ot = op.tile([128, 512], out_dt, tag="ot")
                    nc.vector.tensor_copy(ot[:], po[:])
                    nc.sync.dma_start(out[ts(tg, 128), ts(oi, 512)], ot[:])

            # -------- software-pipelined emission --------
            filler = deque()   # pending PE-heavy units
            proj_q = deque()   # proj tiles ready to emit
            state = {"pv": 0}

            def advance_pipeline(upto, spend_proj=False):
                nl = POLICY["norm_lag"]
                sp = POLICY["pv_split"]
                while state["pv"] <= min(upto, NT128 - 1):
                    tg = state["pv"]
                    if sp:
                        if tg >= 1:
                            if sp == 1:
                                emit_pv_fin(tg - 1)
                            emit_norm(tg - 1)
                        if tg >= 2:
                            emit_transpose(tg - 2)
                        if spend_proj and proj_q:
                            emit_proj(proj_q.popleft())
                        emit_pv(tg, last_ki=(tg - 1 if sp == 1 else tg))
                    else:
                        if nl and tg >= 1:
                            emit_norm(tg - 1)
                        if tg >= 1 + nl:
                            emit_transpose(tg - 1 - nl)
                        if spend_proj and proj_q:
                            emit_proj(proj_q.popleft())
                        emit_pv(tg)
                    state["pv"] += 1

            for ti in range(NT512):
                if ti == 0:
                    for fn, args in qk_units(0):
                        fn(*args)
                    emit_bulk_loads()
                ensure_xt_dma(ti)
                for pf in range(1, POLICY["xt_prefetch"] + 1):
                    ensure_xt_dma(ti + pf)
                if ti == 3 and POLICY["v3_defer"] == 2:
                    pass  # v(3) was queued during stripe 2
                elif POLICY["v_defer_all"] or (ti == 3 and POLICY["v3_defer"]):
                    for tj in range(3, -1, -1):
                        filler.appendleft((emit_v_chain, (ti, tj)))
                else:
                    for tj in range(4):
                        emit_v_chain(ti, tj)
                if ti + 1 < NT512:
                    for unit in qk_units(ti + 1):
                        filler.append(unit)
                    if ti + 1 == 3 and POLICY["v3_defer"] == 2:
                        # v(3) as late stripe-2 filler: frees stripe-3 PE
                        # for the diag endgame
                        for tj in range(4):
                            filler.append((emit_v_chain, (ti + 1, tj)))
                if POLICY["adv_fill"] and ti > 0:
                    units = [(advance_pipeline, (m,))
                             for m in range(state["pv"], 4 * ti)]
                    for u in reversed(units):
                        filler.appendleft(u)
                else:
                    advance_pipeline(4 * ti - 1)

                nk = 4 * ti + 4
                spend = POLICY["spend_proj"]
                nfill = 1 + (POLICY["double_fill"] and ti == 3)
                for ki in range(nk):
                    def _slot_work():
                        if ki - 1 >= 4 * ti:
                            if ti == 0 and filler:
                                fn, args = filler.popleft()
                                fn(*args)
                            advance_pipeline(
                                ki - 1,
                                spend_proj=(spend == "all"
                                            or (spend in ("s3", "s23")
                                                and ti == 3)
                                            or (spend == "s23" and ti == 2)))
                        else:
                            for _ in range(nfill):
                                if filler:
                                    fn, args = filler.popleft()
                                    fn(*args)
                                elif proj_q:
                                    emit_proj(proj_q.popleft())
                    if POLICY["slot_swap"]:
                        _slot_work()
                        emit_scores(ti, ki)
                    else:
                        emit_scores(ti, ki)
                        _slot_work()
                while filler:
                    fn, args = filler.popleft()
                    fn(*args)

            def emit_tail_fine(fin=False):
                # tile 15: per-slab fin -> norm -> transpose -> yT copy
                # interleaved with the proj chain; slab 0 overlaps the g=1
                # exp+mask of the final diagonal
                tg = NT128 - 1
                py4 = py4s[tg]
                y_n = yn.tile([128, HPC, D], bf16, tag="yn")
                yTt = ps_y.tile([128, 2, 128], bf16, tag="py")
                pos = [ps_big.tile([128, 512], f32, tag="big", name=f"po_t{k}")
                       for k in range(2)]
                tg_rel, qi = tg % 4, tg // 4
                for s in range(2):
                    if fin:
                        for h in (2 * s, 2 * s + 1):
                            for ki in range(pv_done[tg] + 1, tg + 1):
                                nc.tensor.matmul(
                                    py4[:, h, :],
                                    p4_all[(qi, ki)][:, h, ts(tg_rel, 128)],
                                    vaug[:, ki, h, :],
                                    start=False,
                                    stop=(h == HPC - 1 and ki == tg),
                                    skip_group_check=True)
                    rec2 = yn.tile([128, 2, 1], f32, tag="rec")
                    nc.vector.reciprocal(
                        rec2[:], py4[:, ts(s, 2), D:D + 1])
                    nc.vector.tensor_mul(
                        out=y_n[:, ts(s, 2), :], in0=py4[:, ts(s, 2), 0:D],
                        in1=rec2.to_broadcast([128, 2, D]))
                    nc.tensor.transpose(
                        yTt[:, s, :], y_n[:, ts(s, 2), :], Id_sb[:])
                    nc.vector.tensor_copy(
                        yT[:, s, ts(tg, 128)], yTt[:, s, :])
                    for oi in range(2):
                        nc.tensor.matmul(
                            pos[oi][:], yT[:, s, ts(tg, 128)],
                            wpt_sb[:, s, ts(oi, 512)],
                            start=(s == 0), stop=(s == 1))
                ot2 = op.tile([128, 2, 512], out_dt, tag="ot2")
                nc.vector.tensor_copy(ot2[:, 0, :], pos[0][:])
                if POLICY["drain_alt"] in (1, 2):
                    nc.scalar.activation(ot2[:, 1, :], pos[1][:], AF.Copy)
                else:
                    nc.vector.tensor_copy(ot2[:, 1, :], pos[1][:])
                nc.sync.dma_start(out[ts(tg, 128), :], ot2[:])

            advance_pipeline(NT128 - 1)
            if POLICY["pv_split"]:
                # state: pv(15) open at ki<=14, norm done <=14, transp <=13
                emit_transpose(NT128 - 2)
                while proj_q:
                    emit_proj(proj_q.popleft(),
                              split_drain=POLICY["drain_alt"] == 2)
                if POLICY["tail_fine"]:
                    emit_tail_fine(fin=True)
                else:
                    emit_pv_fin(NT128 - 1)
                    emit_norm(NT128 - 1)
                    emit_transpose(NT128 - 1)
                    emit_proj(NT128 - 1,
                              split_drain=POLICY["drain_alt"] == 2)
            elif POLICY["norm_lag"]:
                if POLICY["tail_fine"]:
                    emit_transpose(NT128 - 2)
                    while proj_q:
                        emit_proj(proj_q.popleft(),
                                  split_drain=POLICY["drain_alt"] == 2)
                    emit_tail_fine()
                else:
                    emit_norm(NT128 - 1)
                    emit_transpose(NT128 - 2)
            if not POLICY["pv_split"] and \
                    not (POLICY["norm_lag"] and POLICY["tail_fine"]):
                emit_transpose(NT128 - 1)
                while proj_q:
                    emit_proj(proj_q.popleft(),
                              split_drain=POLICY["drain_alt"] == 2)

    nc.compile()
    return nc


def _get_compiled():
    global _COMPILED
    if _COMPILED is None:
        _COMPILED = _build()
    return _COMPILED


def _host_prep(x, W_attn, b_attn, W_proj, b_proj):
    import ml_dtypes
    scale = 1.0 / np.sqrt(np.float32(D))
    xTb = [np.ascontiguousarray(x[b].T).astype(np.float32) for b in range(B)]
    Sm = (np.arange(128, dtype=np.int32)[None, :]
          >= np.arange(128, dtype=np.int32)[:, None]).astype(np.float32)
    Idm = np.eye(128, dtype=np.float32)

    # channel order for a qk chain with sub-row i: psum partition p' holds
    # local channel c = 64*(p'//32) + 32*i + (p'%32)
    pp_ = np.arange(128)
    c_of_p = {i: 64 * (pp_ // 32) + 32 * i + (pp_ % 32) for i in (0, 1)}

    in_maps = []
    for c in range(N_CORES):
        b, g = divmod(c, 4)
        ch = slice(CH * g, CH * (g + 1))
        Wq = W_attn[ch]                    # [256, C]
        Wk = W_attn[C:][ch] * scale
        Wv = W_attn[2 * C:][ch]
        bq = b_attn[ch]
        bk = b_attn[C:][ch] * scale
        bv = b_attn[2 * C:][ch]

        # x8: [128, 4(j), 2(i), T] = xT[(2j+i)*128+p, t] * XS
        x8_c = np.ascontiguousarray(
            (xTb[b].reshape(4, 2, 128, T).transpose(2, 0, 1, 3) * XS)
        ).astype(ml_dtypes.float8_e4m3)

        # xw8: [128(p), 4(j), 2(i_row), 1024] = wqk cols (512) | x8 stripe0
        # chain cc: 0=q,i0 1=q,i1 2=k,i0 3=k,i1; col p' -> channel c_of_p
        wqk = np.empty((C, 4, 128), dtype=np.float32)  # [row, chain, col]
        wqk[:, 0, :] = Wq[c_of_p[0]].T
        wqk[:, 1, :] = Wq[c_of_p[1]].T
        wqk[:, 2, :] = Wk[c_of_p[0]].T
        wqk[:, 3, :] = Wk[c_of_p[1]].T
        wqk = (wqk.reshape(4, 2, 128, 4 * 128).transpose(2, 0, 1, 3)
               * WS).astype(np.float32)
        xw8_c = np.ascontiguousarray(np.concatenate(
            [wqk, x8_c[:, :, :, :512].astype(np.float32)], axis=3)
        ).astype(ml_dtypes.float8_e4m3)

        # bqk: [128, 4] = QS * bias[channel(p', chain)]
        bqk_c = np.empty((128, 4), dtype=np.float32)
        bqk_c[:, 0] = QS * bq[c_of_p[0]]
        bqk_c[:, 1] = QS * bq[c_of_p[1]]
        bqk_c[:, 2] = QS * bk[c_of_p[0]]
        bqk_c[:, 3] = QS * bk[c_of_p[1]]

        bvb_c = np.ascontiguousarray(
            np.broadcast_to(bv[None, :].reshape(1, HPC, D),
                            (128, HPC, D))).astype(np.float32)
        wpt_c = np.ascontiguousarray(
            W_proj[:, ch].T).astype(ml_dtypes.bfloat16)

        im = {
            "x8": x8_c,
            "xw8": xw8_c,
            "wpt": wpt_c,
            "bqk": bqk_c,
            "bvb": bvb_c,
            "Sm": np.ascontiguousarray(np.broadcast_to(
                Sm[:, None, :], (128, 2, 128))).astype(ml_dtypes.bfloat16),
            "Idm": Idm.astype(ml_dtypes.bfloat16),
        }
        if POLICY["v_pair"]:
            # r8: fp8 residual of x at 8x the x8 scale; layout like x8
            x_f = xTb[b].reshape(4, 2, 128, T).transpose(2, 0, 1, 3)
            x8_f = x8_c.astype(np.float32)
            r8_c = np.ascontiguousarray(
                (x_f - x8_f / XS) * RS).astype(ml_dtypes.float8_e4m3)
            # wv8: [128, 4(j), 2(i), 512] = w8 cols (256) | s8 cols (256)
            wvt = Wv.T  # [C, 256]
            w8 = (wvt * WS).astype(ml_dtypes.float8_e4m3)
            s8 = ((wvt - w8.astype(np.float32) / WS) * SS
                  ).astype(ml_dtypes.float8_e4m3)
            wv8_f = np.concatenate(
                [w8.astype(np.float32), s8.astype(np.float32)], axis=1)
            wv8_c = np.ascontiguousarray(
                wv8_f.reshape(4, 2, 128, 2 * CH).transpose(2, 0, 1, 3)
            ).astype(ml_dtypes.float8_e4m3)
            im["r8"] = r8_c
            im["wv8"] = wv8_c
        else:
            im["xT"] = xTb[b].astype(ml_dtypes.bfloat16)
            im["wv"] = np.ascontiguousarray(
                Wv.T.reshape(8, 128, CH).transpose(1, 0, 2)
            ).astype(ml_dtypes.bfloat16)
        in_maps.append(im)
    return in_maps


def kernel(x, W_attn, b_attn, W_proj, b_proj):
    x = np.asarray(x, dtype=np.float32)
    W_attn = np.asarray(W_attn, dtype=np.float32)
    b_attn = np.asarray(b_attn, dtype=np.float32)
    W_proj = np.asarray(W_proj, dtype=np.float32)
    b_proj = np.asarray(b_proj, dtype=np.float32)

    nc = _get_compiled()
    in_maps = _host_prep(x, W_attn, b_attn, W_proj, b_proj)
    res = run_bass_kernel_spmd(nc, in_maps, core_ids=list(range(N_CORES)))

    out = np.empty((B, T, C), dtype=np.float32)
    for b in range(B):
        acc = np.asarray(res.results[4 * b]["out_partial"],
                         dtype=np.float32).copy()
        for g in range(1, 4):
            acc += np.asarray(res.results[4 * b + g]["out_partial"],
                              dtype=np.float32)
        out[b] = acc + b_proj
    return out
